# revision 34
# baseline (speedup 1.0000x reference)
"""Trainium2 Bass kernel for nn_Attention (B=4, N=2048, C=768, H=8).

reference:
    qkv = x.reshape(B,N,H,d).transpose(0,2,1,3)      # q=k=v
    attn = softmax(q @ k^T / sqrt(d))
    out  = (attn @ v).transpose -> (B,N,C)
    y    = out @ proj_w.T + proj_b

Sharding: 8 cores = 4 batches x 2 head-halves. Each core computes the FULL
2048x2048 attention for 4 heads of one batch plus the partial projection
over its 384 input features of the proj matmul; the host sums the two
partial Y's per batch and adds the bias. No device collectives.

Because q = k = v, the score matrix S = X_h X_h^T per head is SYMMETRIC:
only the lower-triangle tiles (ktile >= 4*qchunk) are computed on PE and
exp'd on ACT; the upper tiles are DMA-transposed copies (XBAR transpose on
the SP queue) of already-exp'd tiles.  This cuts mm1 PE work and exp ACT
work to 40/64 per head.

Layout ([feature, token], contraction dim on partitions):
  xt[h]  = X_h^T [96, 2048] bf16 -- doubles as lhsT (k-tiles) and rhs (q)
  S^T[k,q] tile = xt-tile.T-contract @ xt-qslice    (PSUM [128,2,512])
  es[J]  = exp(scale*S^T) bf16 [128, 16kt, 512q] per q-chunk, filled by
           ACT (lower tiles) and by dma_start_transpose (upper tiles)
  po     = [V_h | 1][128kt,97].T @ es[J][:,kt,:] accumulated over 16 kt
           (row 96 = softmax denominator)
  O_norm = po * (1/denom) -> repacked into 3 local c-chunks [128, 512]
  partY^T[j,q] = sum_cc wt[cc,:,jtile].T @ oglob[J][cc]   (bf16 out)

Schedule: per q-chunk J the PE order per 2-ktile group is
  mm1(g) | mm2(g-1, deferred) | mm2 cushion on transposed tiles | proj
so the ACT exp stream is never starved; the 4J transposed ktiles need no
exp and give PE independent cushion work.  Projection units for chunk J
are queued once the last head's chunk J is emitted and drained as PE
fillers from group 2 of later chunks; the final chunk's units run at the
tail c-chunk-major on 6 parallel accumulators (freed score banks + ps_y)
so only the last 6 matmuls wait on the final repack.
PSUM: 2x2 score banks + 2 PV accumulators + 2 proj accumulators = 8.
"""

import sys
import os

for _p in ("/opt/trn_rl_repo",):
    if os.path.isdir(_p) and _p not in sys.path:
        sys.path.insert(0, _p)

from collections import deque

import numpy as np
import ml_dtypes

import concourse.bacc as bacc
import concourse.mybir as mybir
import concourse.tile as tile
from concourse.bass import MemorySpace
from concourse.bass_utils import run_bass_kernel_spmd

BF16 = ml_dtypes.bfloat16

B, N, C = 4, 2048, 768
H = 8
D = C // H            # 96
NCORES = 8
HPC = H // 2          # heads per core = 4
QC = 512              # q chunk (PSUM free size)
NQC = N // QC         # 4 q chunks per head
KT = N // 128         # 16 key tiles
JT = C // 128         # 6 output-feature tiles
CC = HPC * D // 128   # 3 local contraction chunks (384/128)
GS = 2                # ktiles per mm1/exp group
SCALE = float(D) ** -0.5

_cache = {}
DEBUG_LABELS = {}


def _lbl(inst, s):
    name = getattr(getattr(inst, "ins", inst), "name", None)
    if name is not None:
        DEBUG_LABELS[name] = s
    return inst


def build_bass(iters: int = 1):
    """Build the SPMD single-core program (same graph on all 8 cores)."""
    nc = bacc.Bacc("TRN2", target_bir_lowering=False, debug=False,
                   num_devices=NCORES)
    f32 = mybir.dt.float32
    bf16 = mybir.dt.bfloat16

    xt = nc.declare_dram_parameter("xt", [HPC, D, N], bf16, isOutput=False)
    vn = nc.declare_dram_parameter("vn", [HPC, 128, KT, D + 1], bf16,
                                   isOutput=False)
    wt = nc.declare_dram_parameter("wt", [CC, 128, C], bf16, isOutput=False)
    out = nc.declare_dram_parameter("out", [C, N], bf16, isOutput=True)

    with tile.TileContext(nc) as tc:
        with (
            tc.tile_pool(name="consts", bufs=1) as consts,
            tc.tile_pool(name="expp", bufs=1) as expp,
            tc.tile_pool(name="small", bufs=8) as small,
            tc.tile_pool(name="onorm", bufs=NQC * CC + 3) as onormp,
            tc.tile_pool(name="ysb", bufs=8) as ysbp,
            tc.tile_pool(name="ps_s", bufs=2, space=MemorySpace.PSUM) as ps_s,
            tc.tile_pool(name="ps_o", bufs=2, space=MemorySpace.PSUM) as ps_o,
            tc.tile_pool(name="ps_y", bufs=2, space=MemorySpace.PSUM) as ps_y,
        ):
            # ---- load constants (first-needed first) ----
            xt_sb = [consts.tile([D, N], bf16, tag=f"xt{h}", name=f"xt{h}")
                     for h in range(HPC)]
            vn_sb = [consts.tile([128, KT, D + 1], bf16, tag=f"vn{h}",
                                 name=f"vn{h}")
                     for h in range(HPC)]
            wt_sb = [consts.tile([128, C], bf16, tag=f"wt{c}", name=f"wt{c}")
                     for c in range(CC)]
            nc.sync.dma_start(out=xt_sb[0][:, 0:768], in_=xt[0][:, 0:768])
            nc.sync.dma_start(out=xt_sb[0][:, 768:1280],
                              in_=xt[0][:, 768:1280])
            nc.sync.dma_start(out=vn_sb[0][:, 4:8, :], in_=vn[0][:, 4:8, :])
            nc.sync.dma_start(out=xt_sb[0][:, 1280:], in_=xt[0][:, 1280:])
            nc.sync.dma_start(out=vn_sb[0][:, 8:, :], in_=vn[0][:, 8:, :])
            nc.sync.dma_start(out=vn_sb[0][:, 0:4, :], in_=vn[0][:, 0:4, :])
            nc.sync.dma_start(out=xt_sb[1][:], in_=xt[1])
            nc.sync.dma_start(out=vn_sb[1][:], in_=vn[1])
            # h2/h3 tensors and the proj weights are loaded inside the head
            # loop (staggered) so the SP queue isn't backlogged during head
            # 0's transpose burst.

            def load_later(h, J, it):
                if it > 0:
                    return
                if h == 0 and J == 1:
                    nc.sync.dma_start(out=xt_sb[2][:], in_=xt[2])
                elif h == 0 and J == 2:
                    nc.sync.dma_start(out=vn_sb[2][:], in_=vn[2])
                elif h == 1 and J == 1:
                    nc.sync.dma_start(out=xt_sb[3][:], in_=xt[3])
                elif h == 1 and J == 2:
                    nc.sync.dma_start(out=vn_sb[3][:], in_=vn[3])
                elif h == 2 and J < CC:
                    nc.sync.dma_start(out=wt_sb[J][:], in_=wt[J])

            # HAM warmup: dummy matmuls with no input deps keep the PE
            # activity monitor busy during the initial DMA wait so real
            # matmuls start at full clock.
            wz = consts.tile([D, QC], bf16, tag="wz", name="wz")
            nc.vector.memset(wz[:], 0)
            pyw = ps_y.tile([128, QC], f32, tag="py", name="pyw")
            for _w in range(4):
                nc.tensor.matmul(pyw[:], wz[:, 0:128], wz[:],
                                 start=True, stop=True)
            # preload the ScalarE exp table set during the DMA wait
            wze = small.tile([1, 16], bf16, tag="wze", name="wze")
            nc.scalar.activation(out=wze[:], in_=wz[0:1, 0:16],
                                 func=mybir.ActivationFunctionType.Exp)

            pend = [None]        # deferred mm2: (po, cnt, es_J, grp, h, fin)
            cushq = deque()      # ready mm2 matmuls on transposed tiles
            carryq = deque()     # diag-tile mm2s deferred into the next chunk
            projq = deque()      # projection filler units

            def mm2(po, cnt, h, es_J, t_i, fin, tag=""):
                _lbl(nc.tensor.matmul(
                    po[:],
                    vn_sb[h][:, t_i, :],
                    es_J[:, t_i, :],
                    start=(cnt[0] == 0), stop=(cnt[0] == KT - 1),
                ), f"mm2{tag} h?{t_i}")
                cnt[0] += 1
                if cnt[0] == KT:
                    fin()

            def flush_pend():
                w = pend[0]
                if w is None:
                    return
                po, cnt, es_J, grp, h, fin = w
                pend[0] = None
                for t_i in grp:
                    mm2(po, cnt, h, es_J, t_i, fin, tag="P")

            def emit_cushion(k):
                for _ in range(min(k, len(cushq))):
                    cushq.popleft()()

            def emit_carry(k):
                for _ in range(min(k, len(carryq))):
                    carryq.popleft()()

            def emit_proj(k):
                for _ in range(min(k, len(projq))):
                    projq.popleft()()

            def proj_unit(J, j, oglob, py):
                for cc in range(CC):
                    nc.tensor.matmul(
                        py[:],
                        wt_sb[cc][:, j * 128:(j + 1) * 128],
                        oglob[J][cc][:],
                        start=(cc == 0), stop=(cc == CC - 1),
                    )
                y = ysbp.tile([128, QC], bf16, tag="y", name="y")
                nc.vector.tensor_copy(out=y[:], in_=py[:])
                nc.sync.dma_start(
                    out=out[j * 128:(j + 1) * 128, J * QC:(J + 1) * QC],
                    in_=y[:],
                )

            def queue_proj(J, oglob):
                for j in range(JT):
                    def mk(j):
                        def go():
                            py = ps_y.tile([128, QC], f32, tag="py",
                                           name=f"py{J}_{j}")
                            proj_unit(J, j, oglob, py)
                        return go
                    projq.append(mk(j))

            def emit_chunk(h, J, es, oglob, ragged=True):
                assert not cushq
                po = ps_o.tile([D + 1, QC], f32, tag="po")
                cnt = [0]
                es_J = es[J]

                def normalize():
                    rc = small.tile([1, QC], f32, tag="rc")
                    nc.vector.reciprocal(out=rc[:], in_=po[D:D + 1, :])
                    bc = small.tile([D, QC], f32, tag="bc")
                    nc.gpsimd.partition_broadcast(bc[:], rc[:])
                    on = small.tile([D, QC], bf16, tag="on")
                    nc.vector.tensor_mul(on[:], po[0:D, :], bc[:])
                    # repack head rows 96h..96h+96 into the local 128-row
                    # c-chunk layout (DVE cannot shift partitions)
                    a0 = (D * h) % 128
                    c0 = (D * h) // 128
                    s1 = min(128 - a0, D)
                    nc.sync.dma_start(out=oglob[J][c0][a0:a0 + s1, :],
                                      in_=on[0:s1, :])
                    if s1 < D:
                        nc.sync.dma_start(out=oglob[J][c0 + 1][0:D - s1, :],
                                          in_=on[s1:D, :])

                for t_i in range(4 * J):
                    def mk_cush(t_i):
                        def go():
                            mm2(po, cnt, h, es_J, t_i, normalize, tag="C")
                        return go
                    cushq.append(mk_cush(t_i))

                lo = 4 * J
                n_groups = (KT - lo) // GS
                # Diagonal 512-block FIRST: its ragged rows (row lo+r
                # computes only cols [128r, 512)) produce the intra-block
                # mirrors whose consumers (this chunk's diag mm2s, deferred
                # into the next chunk via carryq) are the earliest; issuing
                # them first gives the XBAR transposes a full chunk of
                # latency slack.  Transposable full tiles follow, ascending
                # so mirrors for the next chunk's late cushions go first.
                order = list(range(lo + 4, KT)) + list(range(lo, lo + 4))
                diag_groups = (set(range(n_groups - 2, n_groups))
                               if ragged else set())
                cush_per_group = [0] * n_groups
                for g in range(4 * J):
                    cush_per_group[n_groups - 1 - (g % n_groups)] += 1
                carry_per_group = [0] * n_groups
                for g in range(len(carryq)):
                    carry_per_group[min(g // 2, n_groups - 1)] += 1

                for gi in range(n_groups):
                    grp = order[gi * GS:(gi + 1) * GS]
                    is_diag = gi in diag_groups
                    ps = ps_s.tile([128, GS, QC], f32, tag="ps")
                    q0s = []
                    for i, t_i in enumerate(grp):
                        q0 = 128 * (t_i - lo) if is_diag else 0
                        q0s.append(q0)
                        _lbl(nc.tensor.matmul(
                            ps[:, i, q0:],
                            xt_sb[h][:, t_i * 128:(t_i + 1) * 128],
                            xt_sb[h][:, J * QC + q0:(J + 1) * QC],
                            start=True, stop=True,
                        ), f"mm1 h{h} J{J} t{t_i}")
                    emit_carry(carry_per_group[gi])
                    flush_pend()
                    emit_cushion(cush_per_group[gi])
                    if gi >= 2:
                        emit_proj(2)
                    if is_diag:
                        for i, t_i in enumerate(grp):
                            _lbl(nc.scalar.activation(
                                out=es_J[:, t_i, q0s[i]:],
                                in_=ps[:, i, q0s[i]:],
                                func=mybir.ActivationFunctionType.Exp,
                                scale=SCALE,
                            ), f"exp h{h} J{J} d{t_i}")
                            # mirror this row's computed sub-blocks into the
                            # missing lower-cols of later diag rows
                            r = t_i - lo
                            for r2 in range(r + 1, 4):
                                _lbl(nc.sync.dma_start_transpose(
                                    out=es_J[:, lo + r2,
                                             128 * r:128 * (r + 1)],
                                    in_=es_J[:, t_i,
                                             128 * r2:128 * (r2 + 1)],
                                ), f"trd h{h} J{J} r{r}->r{r2}")
                        # diag mm2s ride the carry queue into the next chunk
                        for i, t_i in enumerate(grp):
                            def mk_carry(t_i):
                                def go():
                                    mm2(po, cnt, h, es_J, t_i, normalize,
                                        tag="D")
                                return go
                            carryq.append(mk_carry(t_i))
                    else:
                        es_w = es_J[:, grp[0]:grp[0] + GS, :]
                        _lbl(nc.scalar.activation(
                            out=es_w, in_=ps[:, 0:GS, :],
                            func=mybir.ActivationFunctionType.Exp,
                            scale=SCALE,
                        ), f"exp h{h} J{J} g{grp[0]}-{grp[-1]}")
                        for t_i in grp:
                            if t_i >= 4 * (J + 1):
                                q0 = 128 * (t_i % 4)
                                _lbl(nc.sync.dma_start_transpose(
                                    out=es[t_i // 4][:, 4 * J:4 * J + 4,
                                                     q0:q0 + 128],
                                    in_=es_J[:, t_i, :],
                                ), f"tr h{h} J{J} t{t_i}->es{t_i // 4}")
                        pend[0] = (po, cnt, es_J, grp, h, normalize)
                assert not cushq

            for it in range(iters):
                oglob = {J: [onormp.tile([128, QC], bf16, tag="og",
                                         name=f"og{J}_{c}")
                             for c in range(CC)]
                         for J in range(NQC)}
                for h in range(HPC):
                    es = [expp.tile([128, KT, QC], bf16, tag=f"es{J}",
                                    name=f"es{J}")
                          for J in range(NQC)]
                    for J in range(NQC):
                        if h == HPC - 1 and J >= 1:
                            queue_proj(J - 1, oglob)
                        last_chunk = (it == iters - 1 and h == HPC - 1
                                      and J == NQC - 1)
                        emit_chunk(h, J, es, oglob, ragged=not last_chunk)
                        load_later(h, J, it)
                if it < iters - 1:
                    # roll the last chunk's projection into the next iter
                    queue_proj(NQC - 1, oglob)
                else:
                    # tail: flush the final normalize, drain leftovers, then
                    # run the last chunk's projection c-chunk-major on 6
                    # parallel accumulators so only the final 6 matmuls wait
                    # on the last repack.
                    while carryq:
                        emit_carry(8)
                    flush_pend()
                    while projq:
                        emit_proj(8)
                    accs = [ps_s.tile([128, GS, QC], f32, tag="ps",
                                      name=f"acc{a}") for a in range(2)]
                    pys = [ps_y.tile([128, QC], f32, tag="py",
                                     name=f"pyt{a}") for a in range(2)]
                    Jf = NQC - 1

                    def acc_of(j):
                        if j < 4:
                            return accs[j // 2][:, j % 2, :]
                        return pys[j - 4][:]

                    for cc in range(CC):
                        for j in range(JT):
                            nc.tensor.matmul(
                                acc_of(j),
                                wt_sb[cc][:, j * 128:(j + 1) * 128],
                                oglob[Jf][cc][:],
                                start=(cc == 0), stop=(cc == CC - 1),
                            )
                    qs = [nc.sync, nc.gpsimd, nc.scalar]
                    cps = [nc.vector.tensor_copy, nc.scalar.copy]
                    for j in range(JT):
                        y = ysbp.tile([128, QC], bf16, tag="y", name=f"yf{j}")
                        cps[j % 2](out=y[:], in_=acc_of(j))
                        qs[j % 3].dma_start(
                            out=out[j * 128:(j + 1) * 128,
                                    Jf * QC:(Jf + 1) * QC],
                            in_=y[:],
                        )
            flush_pend()
    nc.compile()
    return nc


def shard_inputs(x, proj_w, proj_b):
    x = np.asarray(x, dtype=np.float32)
    proj_w = np.asarray(proj_w, dtype=np.float32)

    wt_t = np.ascontiguousarray(proj_w.T)          # [c_in, j]
    in_maps = []
    for core in range(NCORES):
        b = core // 2
        s = core % 2
        xb = x[b]                                  # (N, C)
        xtb = np.ascontiguousarray(xb.T)           # (C, N)
        xt_c = xtb.reshape(H, D, N)[4 * s:4 * s + 4].astype(BF16)
        # vn[h, p, kt, d] = x[b, 128*kt+p, 96*(4s+h)+d], ones at d=96
        vnf = np.ones((HPC, 128, KT, D + 1), dtype=np.float32)
        xr = xb.reshape(KT, 128, H, D)             # [kt, p, h, d]
        vnf[:, :, :, :D] = xr[:, :, 4 * s:4 * s + 4, :].transpose(2, 1, 0, 3)
        wt_c = np.ascontiguousarray(
            wt_t[384 * s:384 * s + 384]).reshape(CC, 128, C).astype(BF16)
        in_maps.append({
            "xt": np.ascontiguousarray(xt_c),
            "vn": vnf.astype(BF16),
            "wt": wt_c,
        })
    return in_maps


def assemble(results, proj_b):
    y = np.empty((B, N, C), dtype=np.float32)
    for b in range(B):
        p0 = results[2 * b]["out"].astype(np.float32)
        p1 = results[2 * b + 1]["out"].astype(np.float32)
        y[b] = (p0 + p1).T + proj_b
    return y


def kernel(x, proj_w, proj_b):
    proj_b = np.asarray(proj_b, dtype=np.float32)
    if "nc" not in _cache:
        _cache["nc"] = build_bass()
    nc = _cache["nc"]
    in_maps = shard_inputs(x, proj_w, proj_b)
    res = run_bass_kernel_spmd(nc, in_maps, core_ids=list(range(NCORES)))
    return assemble(res.results, proj_b)
